# revision 7
# baseline (speedup 1.0000x reference)
"""Trainium2 Bass kernel for BaseBidirectionalAttention (fused-linear version).

Problem shapes (hardcoded): B=32, C=1024, Q=128, D=256, F=4D=1024.
Sharding: data-parallel over batch across 8 cores (4 batch elems/core);
weights replicated.

Algebraic restructurings vs the reference (all exact in real arithmetic):
  1. Fused linears: masking is row-wise and there is no nonlinearity between
     the two linears, so
       relu(((att@W1.T+b1)*m @ W2.T + b2)*m) = relu((att@W12.T + b12)*m)
     with W12 = W2@W1, b12 = W2@b1 + b2 precomputed on host.  Halves the
     dominant matmul work.
  2. att = [ctx, c2q, ctx*c2q, ctx*q2c]; q2c is constant over context rows,
     so the ctx and ctx*q2c pieces merge via a per-elem scaled weight block
     W_eff = A + D*diag(q2c)  (DVE prep, no extra matmul k-steps).
  3. c2q = P @ question (P = softmax(sim) over q), so
     c2q @ B.T = P @ (question @ B.T) = P @ QB  -- contraction 256 -> 128.
     Softmax rows sum to 1, so the bias rides along free: QB' = QB + b12.
  4. cwc = ctx.w_context folds into the sim matmul as a 129th moving column;
     qwq = question.w_question folds in as a K=1 accumulating matmul row.
     (cwc is constant over q so it cancels in softmax-q; qwq is needed in the
     logits only for the max-over-q used by the q2c path.)

Per-core per-elem device program (natural-layout output):
  sim(C,129)  = ctxT16.T @ [q*wm | w_c]  (+ qwq via K=1 row)      PE fp16
  P(C,Q)      = softmax_q(sim[:, :128])                           DVE/ACT
  PT(Q,C), cxc=(ctx*c2q)^T, QB'=q@B.T+b12, W_eff=A+D*diag(q2c)
  out(C,F)    = relu((ctx@W_eff.T + cxc.T@C.T + P@QB') * m)       PE fp16

Heavy matmuls run fp16 (1 cyc/row, separate hoistable LDWEIGHTS + FWL;
fp32r is self-loading and 4 cyc/row under N=256).  Softmax statistics, exp,
q2c weighting and all PSUM accumulation stay fp32.  Output is stored fp16
(halves the dominant DMA stream) and upcast on host.
"""

import sys

if "/opt/trn_rl_repo" not in sys.path:
    sys.path.insert(0, "/opt/trn_rl_repo")

import numpy as np

import concourse.bass as bass
import concourse.mybir as mybir
import concourse.tile as tile
from concourse import bacc
from concourse.bass_utils import run_bass_kernel_spmd
from concourse.masks import make_identity

B, C, Q, D = 32, 1024, 128, 256
F = 4 * D
NCORES = 8
BPC = B // NCORES  # batch elems per core
P = 128
CT = C // P   # 8 c-tiles
FT = F // P   # 8 f-tiles
DH = D // P   # 2 halves of D

FP32 = mybir.dt.float32
FP32R = mybir.dt.float32r
FP16 = mybir.dt.float16
AX = mybir.AxisListType.X
AF = mybir.ActivationFunctionType


def _build_body(es, tc, outs, ins, n_elems=BPC, reps=1):
    nc = tc.nc
    ctx_d, qst_d, vecsT_d, w12t_d, b12r_d, mT_d = ins
    out_d = outs[0]

    const = es.enter_context(tc.tile_pool(name="const", bufs=1))
    weights = es.enter_context(tc.tile_pool(name="weights", bufs=1))
    loads = es.enter_context(tc.tile_pool(name="loads", bufs=3))
    work = es.enter_context(tc.tile_pool(name="work", bufs=1))
    outp = es.enter_context(tc.tile_pool(name="outp", bufs=4))
    psA = es.enter_context(tc.tile_pool(name="psA", bufs=5, space="PSUM"))
    psB = es.enter_context(tc.tile_pool(name="psB", bufs=3, space="PSUM"))

    # ---- constants / replicated weights ----
    ident = const.tile([P, P], FP32)
    make_identity(nc, ident)
    ident16 = const.tile([P, P], FP16)
    make_identity(nc, ident16)
    ones_row = const.tile([1, P], FP32)
    nc.vector.memset(ones_row, 1.0)
    ones16 = const.tile([1, P], FP16)
    nc.vector.memset(ones16, 1.0)

    def load_elem(b, idx):
        cn = loads.tile([P, CT, D + 1], FP32, tag="cn", name=f"cn{idx}")
        src_ap = ctx_d[b].rearrange("(t p) d -> p t d", p=P)
        half = CT // 2
        nc.sync.dma_start(cn[:, :half, 0:D], src_ap[:, :half])
        nc.sync.dma_start(cn[:, half:, 0:D], src_ap[:, half:])
        nc.vector.memset(cn[:, :, D:D + 1], 1.0)  # ones col: q2c denominator
        qn = loads.tile([P, D], FP32, tag="qn", name=f"qn{idx}")
        nc.sync.dma_start(qn[:], qst_d[b])
        return cn, qn

    # elem-0 loads go before the big weight DMAs (single-shot only: with a
    # For_i timing loop the hoisted tile's slot would be recycled in-loop)
    pend = load_elem(0, 0) if reps == 1 else None

    vecsT = const.tile([P, DH, 3], FP32)  # [p, h, v]: wq/wc/wm at e=h*128+p
    nc.sync.dma_start(vecsT[:], vecsT_d.rearrange("(h p) v -> p h v", p=P))
    vecs16 = const.tile([P, DH, 4], FP16)
    nc.vector.tensor_copy(vecs16[:, :, 0:3], vecsT[:])

    w12t16 = weights.tile([P, FT, F], FP16)  # [fl, k, f'] = W12[f', k*128+fl]
    nc.sync.dma_start(w12t16[:], w12t_d.rearrange("(k p) f -> p k f", p=P))
    b12bc16 = const.tile([P, F], FP16)  # b12 broadcast to all partitions
    nc.gpsimd.dma_start(
        out=b12bc16[:],
        in_=bass.AP(tensor=b12r_d.tensor, offset=b12r_d.offset,
                    ap=[[0, P]] + b12r_d.ap[1:]),
    )
    mT = const.tile([P, n_elems * CT], FP32)  # [p, b*8+t] = mask[b, t*128+p]
    nc.sync.dma_start(mT[:], mT_d)

    if reps > 1:
        es.enter_context(tc.For_i(0, reps, 1))

    def preamble(cn, qn, idx):
        """Transposes + fp16 prep: ctxT16, qst16, qstT16, qmx16, qwqx16."""
        ctxT16 = work.tile([P, DH, C], FP16, tag="ctxT16", bufs=2,
                           name=f"ctxT16_{idx}")
        for dh in range(DH):
            for g in range(2):  # two groups of 4 c-tiles -> one psum bank
                pt = psA.tile([P, 512], FP32, tag="ps_mm", name=f"ptc{idx}{dh}{g}")
                for j in range(4):
                    t = g * 4 + j
                    nc.tensor.transpose(
                        pt[:, j * P:(j + 1) * P],
                        cn[:, t, dh * P:(dh + 1) * P],
                        ident[:],
                    )
                nc.scalar.copy(ctxT16[:, dh, g * 512:(g + 1) * 512], pt[:])

        qst16 = work.tile([P, D], FP16, tag="qst16", bufs=2, name=f"qst16_{idx}")
        nc.vector.tensor_copy(qst16[:], qn[:])

        pq = psB.tile([P, 2 * P], FP32, tag="ps_small", name=f"pq{idx}")
        for dh in range(DH):
            nc.tensor.transpose(pq[:, dh * P:(dh + 1) * P],
                                qn[:, dh * P:(dh + 1) * P], ident[:])
        qstT16 = work.tile([P, DH, P], FP16, tag="qstT16", bufs=2,
                           name=f"qstT16_{idx}")
        nc.vector.tensor_copy(qstT16[:].rearrange("p h q -> p (h q)"), pq[:])

        # moving operand of sim: [q*wm | w_c], padded to 136 for alignment
        qmx16 = work.tile([P, DH, Q + 8], FP16, tag="qmx16", bufs=2,
                          name=f"qmx16_{idx}")
        for dh in range(DH):
            nc.vector.tensor_scalar_mul(qmx16[:, dh, 0:Q], qstT16[:, dh, :],
                                        vecsT[:, dh, 2:3])
            nc.vector.tensor_copy(qmx16[:, dh, Q:Q + 1], vecsT[:, dh, 1:2])

        # qwq row (K=1 accumulate into sim); col Q is 0 so cwc stays clean
        pw = psB.tile([1, P], FP32, tag="ps_small", name=f"pw{idx}")
        for dh in range(DH):
            nc.tensor.matmul(pw[:], vecs16[:, dh, 0:1], qstT16[:, dh, :],
                             start=(dh == 0), stop=(dh == DH - 1))
        qwqx16 = work.tile([1, Q + 8], FP16, tag="qwqx16", bufs=2,
                           name=f"qwqx16_{idx}")
        nc.vector.tensor_copy(qwqx16[0:1, 0:Q], pw[:])
        nc.vector.memset(qwqx16[0:1, Q:Q + 1], 0.0)
        return ctxT16, qst16, qstT16, qmx16, qwqx16

    pre = None
    for b in range(n_elems):
        # ---- loads (elem b prefetched; prefetch b+1 now) ----
        if pend is None:
            pend = load_elem(b, b)
        cn, qn = pend
        pend = load_elem(b + 1, b + 1) if b + 1 < n_elems else None
        if pre is None:
            pre = preamble(cn, qn, b)
        ctxT16, qst16, qstT16, qmx16, qwqx16 = pre

        # fp16 natural-layout ctx (incl. ones col) for the q2c numerator;
        # ACT-side copy so it overlaps PE work
        cnat16 = work.tile([P, CT, D + 4], FP16, tag="cnat16", bufs=2,
                           name=f"cnat16_{b}")
        nc.scalar.copy(cnat16[:, :, 0:D + 1], cn[:, :, 0:D + 1])

        # ---- sim tiles + softmax over q (free dim) ----
        nmx = work.tile([P, CT], FP32, tag="nmx")    # negated row max
        sume = work.tile([P, CT], FP32, tag="sume")
        rs = work.tile([P, CT], FP32, tag="rs")
        pcwc = work.tile([P, CT], FP32, tag="pcwc")  # cwc columns [c_l, t]
        Pm16 = work.tile([P, CT, Q], FP16, tag="Pm16")
        for t in range(CT):
            ps = psB.tile([P, Q + 1], FP32, tag="ps_small")
            for dh in range(DH):
                nc.tensor.matmul(
                    ps[:], ctxT16[:, dh, t * P:(t + 1) * P],
                    qmx16[:, dh, 0:Q + 1],
                    start=(dh == 0), stop=False,
                )
            nc.tensor.matmul(ps[:], ones16[:], qwqx16[:, 0:Q + 1],
                             start=False, stop=True)
            nc.vector.reduce_max(nmx[:, t:t + 1], ps[:, 0:Q], axis=AX,
                                 negate=True)
            nc.scalar.activation(
                Pm16[:, t, :], ps[:, 0:Q], AF.Exp, bias=nmx[:, t:t + 1],
                accum_out=sume[:, t:t + 1],
            )
            nc.vector.tensor_copy(pcwc[:, t:t + 1], ps[:, Q:Q + 1])
            nc.vector.reciprocal(rs[:, t:t + 1], sume[:, t:t + 1])
            nc.vector.tensor_scalar_mul(Pm16[:, t, :], Pm16[:, t, :],
                                        rs[:, t:t + 1])

        # ---- PT, c2q -> cxc, QB' ----
        PT16 = work.tile([P, C], FP16, tag="PT16")   # [q, c]
        cxc16 = work.tile([P, DH, C], FP16, tag="cxc16")
        for g in range(2):
            pt16 = psA.tile([P, 512], FP16, tag="ps_mm", name=f"pt16{g}")
            for j in range(4):
                t = g * 4 + j
                nc.tensor.transpose(pt16[:, j * P:(j + 1) * P], Pm16[:, t, :],
                                    ident16[:])
            nc.vector.tensor_copy(PT16[:, g * 512:(g + 1) * 512], pt16[:])
            for dh in range(DH):
                pc2 = psA.tile([P, 512], FP32, tag="ps_mm", name=f"pc2{g}{dh}")
                nc.tensor.matmul(
                    pc2[:], qst16[:, dh * P:(dh + 1) * P],
                    PT16[:, g * 512:(g + 1) * 512],
                    start=True, stop=True,
                )
                nc.vector.tensor_mul(cxc16[:, dh, g * 512:(g + 1) * 512],
                                     ctxT16[:, dh, g * 512:(g + 1) * 512],
                                     pc2[:])
        QB16 = work.tile([P, F], FP16, tag="QB16")   # [q, f'] = q@B.T + b12
        for fh in range(2):
            pqb = psA.tile([P, 512], FP32, tag="ps_mm", name=f"pqb{fh}")
            for dh in range(DH):
                nc.tensor.matmul(
                    pqb[:], qstT16[:, dh, :],
                    w12t16[:, 2 + dh, fh * 512:(fh + 1) * 512],
                    start=(dh == 0), stop=(dh == DH - 1),
                )
            nc.vector.tensor_add(QB16[:, fh * 512:(fh + 1) * 512], pqb[:],
                                 b12bc16[:, fh * 512:(fh + 1) * 512])

        # ---- q2c weights + W_eff = A + D*diag(q2c) ----
        madj = work.tile([P, CT], FP32, tag="madj")  # m_c = cwc - nmx
        nc.vector.tensor_sub(madj[:], pcwc[:], nmx[:])
        colmin = work.tile([P, 1], FP32, tag="colmin")
        nc.vector.reduce_max(colmin[:], madj[:], axis=AX, negate=True)
        pcm = psB.tile([1, P], FP32, tag="ps_small")
        nc.tensor.transpose(pcm[:], colmin[:], ident[:])
        minall = work.tile([1, 2], FP32, tag="minall")
        nc.vector.tensor_reduce(minall[:, 0:1], pcm[:], axis=AX,
                                op=mybir.AluOpType.min)
        pmb = psB.tile([P, 1], FP32, tag="ps_small")
        nc.tensor.matmul(pmb[:], ones_row[:], minall[:, 0:1], start=True,
                         stop=True)
        minb = work.tile([P, 1], FP32, tag="minb")
        nc.vector.tensor_copy(minb[:], pmb[:])
        wall16 = work.tile([P, CT], FP16, tag="wall16")  # exp(m - Mglob)
        nc.scalar.activation(wall16[:], madj[:], AF.Exp, bias=minb[:])

        # numerator row [1, 256] + denominator (ones col of cn) in one group
        pn = psB.tile([1, D + 1], FP32, tag="ps_small", name="pn")
        for t in range(CT):
            nc.tensor.matmul(pn[:], wall16[:, t:t + 1], cnat16[:, t, 0:D + 1],
                             start=(t == 0), stop=(t == CT - 1))
        rden = work.tile([1, 1], FP32, tag="rden")
        nc.vector.reciprocal(rden[:], pn[0:1, D:D + 1])
        q2cr = work.tile([1, D], FP32, tag="q2cr")
        nc.vector.tensor_scalar_mul(q2cr[:], pn[0:1, 0:D], rden[:])
        q2cc = work.tile([P, DH], FP32, tag="q2cc")  # [d_l, dh]
        for dh in range(DH):
            pq2 = psB.tile([P, 1], FP32, tag="ps_small", name=f"pq2{dh}")
            nc.tensor.transpose(pq2[:], q2cr[0:1, dh * P:(dh + 1) * P],
                                ident[0:1, 0:1])
            nc.vector.tensor_copy(q2cc[:, dh:dh + 1], pq2[:])
        weff16 = work.tile([P, DH, F], FP16, tag="weff16")
        for k in range(DH):
            nc.vector.tensor_scalar_mul(weff16[:, k, :], w12t16[:, 6 + k, :],
                                        q2cc[:, k:k + 1])
            nc.vector.tensor_add(weff16[:, k, :], weff16[:, k, :],
                                 w12t16[:, k, :])

        # next elem's preamble: PE transposes run between this elem's fused
        # matmuls; its DVE/ACT copies overlap them
        pre = preamble(pend[0], pend[1], b + 1) if pend is not None else None

        # ---- fused layer (natural layout) + relu*mask + store ----
        for ct in range(CT):
            sl = slice(ct * P, (ct + 1) * P)
            pieces = [
                (ctxT16[:, 0, sl], weff16[:, 0, :]),
                (ctxT16[:, 1, sl], weff16[:, 1, :]),
                (cxc16[:, 0, sl], w12t16[:, 4, :]),
                (cxc16[:, 1, sl], w12t16[:, 5, :]),
                (PT16[:, sl], QB16[:]),
            ]
            p2 = [psA.tile([P, 512], FP32, tag="ps_mm", name=f"p2{ct}{fh}")
                  for fh in range(2)]
            npc = len(pieces)
            for i, (lh, rh) in enumerate(pieces):
                for fh in range(2):
                    nc.tensor.matmul(
                        p2[fh][:], lh, rh[:, fh * 512:(fh + 1) * 512],
                        start=(i == 0), stop=(i == npc - 1),
                    )
            osb = outp.tile([P, F], FP16, tag="osb")
            for fh in range(2):
                nc.scalar.activation(
                    osb[:, fh * 512:(fh + 1) * 512], p2[fh][:], AF.Relu,
                    scale=mT[:, b * CT + ct:b * CT + ct + 1],
                )
            nc.sync.dma_start(out_d[b, sl, :], osb[:])


_NC_CACHE = {}


def _build_nc(n_elems=BPC, reps=1):
    key = (n_elems, reps)
    if key in _NC_CACHE:
        return _NC_CACHE[key]
    nc = bacc.Bacc("TRN2", target_bir_lowering=False, debug=False,
                   num_devices=NCORES)
    ins = [
        nc.dram_tensor("ctx", (n_elems, C, D), FP32, kind="ExternalInput").ap(),
        nc.dram_tensor("qst", (n_elems, Q, D), FP32, kind="ExternalInput").ap(),
        nc.dram_tensor("vecsT", (D, 3), FP32, kind="ExternalInput").ap(),
        nc.dram_tensor("w12t", (F, F), FP16, kind="ExternalInput").ap(),
        nc.dram_tensor("b12r", (1, F), FP16, kind="ExternalInput").ap(),
        nc.dram_tensor("mT", (P, n_elems * CT), FP32, kind="ExternalInput").ap(),
    ]
    outs = [nc.dram_tensor("out", (n_elems, C, F), FP16,
                           kind="ExternalOutput").ap()]
    from contextlib import ExitStack
    with tile.TileContext(nc) as tc, ExitStack() as es:
        _build_body(es, tc, outs, ins, n_elems=n_elems, reps=reps)
    nc.compile()
    _NC_CACHE[key] = (nc, ins, outs)
    return _NC_CACHE[key]


def _host_prep(context, question, context_mask, w_question, w_context,
               w_multiple, W1, b1, W2, b2):
    """Build the 8 per-core input maps from full inputs."""
    context = np.asarray(context, np.float32)
    question = np.asarray(question, np.float32)
    maskf = np.asarray(context_mask).astype(np.float32)
    W1f = np.asarray(W1, np.float32)
    W2f = np.asarray(W2, np.float32)
    W12 = W2f @ W1f
    b12 = W2f @ np.asarray(b1, np.float32) + np.asarray(b2, np.float32)
    vecsT = np.ascontiguousarray(
        np.stack([w_question, w_context, w_multiple]).T.astype(np.float32))
    w12t16 = np.ascontiguousarray(W12.T.astype(np.float16))  # [f, f']
    b12r16 = b12.reshape(1, F).astype(np.float16)
    in_maps = []
    for i in range(NCORES):
        sl = slice(BPC * i, BPC * (i + 1))
        mTc = np.ascontiguousarray(
            maskf[sl].reshape(BPC, CT, P).transpose(2, 0, 1).reshape(P, BPC * CT))
        in_maps.append({
            "ctx": np.ascontiguousarray(context[sl]),
            "qst": np.ascontiguousarray(question[sl]),
            "vecsT": vecsT,
            "w12t": w12t16,
            "b12r": b12r16,
            "mT": mTc,
        })
    return in_maps


def kernel(context, question, context_mask, w_question, w_context, w_multiple,
           W1, b1, W2, b2):
    nc, _, _ = _build_nc()
    in_maps = _host_prep(context, question, context_mask, w_question,
                         w_context, w_multiple, W1, b1, W2, b2)
    res = run_bass_kernel_spmd(nc, in_maps, list(range(NCORES))).results
    out = np.concatenate([res[i]["out"] for i in range(NCORES)], axis=0)
    return out.astype(np.float32)


# revision 17
# speedup vs baseline: 1.2497x; 1.2497x over previous
"""Trainium2 Bass kernel for BaseBidirectionalAttention (fused-linear version).

Problem shapes (hardcoded): B=32, C=1024, Q=128, D=256, F=4D=1024.
Sharding: data-parallel over batch across 8 cores (4 batch elems/core);
weights replicated.

Algebraic restructurings vs the reference (all exact in real arithmetic):
  1. Fused linears: masking is row-wise and there is no nonlinearity between
     the two linears, so
       relu(((att@W1.T+b1)*m @ W2.T + b2)*m) = relu((att@W12.T + b12)*m)
     with W12 = W2@W1, b12 = W2@b1 + b2 precomputed on host.  Halves the
     dominant matmul work.
  2. att = [ctx, c2q, ctx*c2q, ctx*q2c]; q2c is constant over context rows,
     so the ctx and ctx*q2c pieces merge via a per-elem scaled weight block
     W_eff = A + D*diag(q2c)  (DVE prep, no extra matmul k-steps).
  3. c2q = P @ question (P = softmax(sim) over q), so
     c2q @ B.T = P @ (question @ B.T) = P @ QB  -- contraction 256 -> 128.
     Softmax rows sum to 1, so the bias rides along free: QB' = QB + b12.
  4. cwc = ctx.w_context folds into the sim matmul as a 129th moving column;
     qwq = question.w_question folds in as a K=1 accumulating matmul row.
     (cwc is constant over q so it cancels in softmax-q; qwq is needed in the
     logits only for the max-over-q used by the q2c path.)

Per-core per-elem device program (natural-layout output):
  sim(C,129)  = ctxT16.T @ [q*wm | w_c]  (+ qwq via K=1 row)      PE fp16
  P(C,Q)      = softmax_q(sim[:, :128])                           DVE/ACT
  PT(Q,C), cxc=(ctx*c2q)^T, QB'=q@B.T+b12, W_eff=A+D*diag(q2c)
  out(C,F)    = relu((ctx@W_eff.T + cxc.T@C.T + P@QB') * m)       PE fp16

Heavy matmuls run fp16 (1 cyc/row, separate hoistable LDWEIGHTS + FWL;
fp32r is self-loading and 4 cyc/row under N=256).  Softmax statistics, exp,
q2c weighting and all PSUM accumulation stay fp32.  Output is stored fp16
(halves the dominant DMA stream) and upcast on host.
"""

import sys

if "/opt/trn_rl_repo" not in sys.path:
    sys.path.insert(0, "/opt/trn_rl_repo")

import numpy as np

import concourse.bass as bass
import concourse.mybir as mybir
import concourse.tile as tile
from concourse import bacc
from concourse.bass_utils import run_bass_kernel_spmd
from concourse.masks import make_identity

B, C, Q, D = 32, 1024, 128, 256
F = 4 * D
NCORES = 8
BPC = B // NCORES  # batch elems per core
P = 128
CT = C // P   # 8 c-tiles
FT = F // P   # 8 f-tiles
DH = D // P   # 2 halves of D

FP32 = mybir.dt.float32
FP32R = mybir.dt.float32r
FP16 = mybir.dt.float16
AX = mybir.AxisListType.X
AF = mybir.ActivationFunctionType


def _build_body(es, tc, outs, ins, n_elems=BPC, reps=1):
    nc = tc.nc
    ctx_d, qst_d, vecsT_d, w12t_d, b12r_d, mT_d = ins
    out_d = outs[0]

    const = es.enter_context(tc.tile_pool(name="const", bufs=1))
    weights = es.enter_context(tc.tile_pool(name="weights", bufs=1))
    loads = es.enter_context(tc.tile_pool(name="loads", bufs=3))
    work = es.enter_context(tc.tile_pool(name="work", bufs=1))
    outp = es.enter_context(tc.tile_pool(name="outp", bufs=4))
    psA = es.enter_context(tc.tile_pool(name="psA", bufs=5, space="PSUM"))
    psB = es.enter_context(tc.tile_pool(name="psB", bufs=3, space="PSUM"))

    # ---- constants / replicated weights ----
    ident = const.tile([P, P], FP32)
    make_identity(nc, ident)
    ident16 = const.tile([P, P], FP16)
    make_identity(nc, ident16)
    ones_row = const.tile([1, P], FP32)
    nc.vector.memset(ones_row, 1.0)
    ones16 = const.tile([1, P], FP16)
    nc.vector.memset(ones16, 1.0)

    def load_elem(b, idx):
        cn = loads.tile([P, CT, D + 1], FP32, tag="cn", name=f"cn{idx}")
        src_ap = ctx_d[b].rearrange("(t p) d -> p t d", p=P)
        half = CT // 2
        nc.sync.dma_start(cn[:, :half, 0:D], src_ap[:, :half])
        nc.sync.dma_start(cn[:, half:, 0:D], src_ap[:, half:])
        nc.vector.memset(cn[:, :, D:D + 1], 1.0)  # ones col: q2c denominator
        qn = loads.tile([P, D], FP32, tag="qn", name=f"qn{idx}")
        nc.sync.dma_start(qn[:], qst_d[b])
        # fp16 natural ctx (incl. ones col): q2c numerator rhs + transpose
        # source; on the otherwise-idle Pool engine
        cnat16 = loads.tile([P, CT, D + 4], FP16, tag="cnat16",
                            name=f"cnat16_{idx}")
        nc.gpsimd.tensor_copy(cnat16[:, :, 0:D + 1], cn[:, :, 0:D + 1])
        return cn, qn, cnat16

    # elem-0 loads go before the big weight DMAs (single-shot only: with a
    # For_i timing loop the hoisted tile's slot would be recycled in-loop)
    pend = load_elem(0, 0) if reps == 1 else None

    vecsT = const.tile([P, DH, 3], FP32)  # [p, h, v]: wq/wc/wm at e=h*128+p
    nc.sync.dma_start(vecsT[:], vecsT_d.rearrange("(h p) v -> p h v", p=P))
    vecs16 = const.tile([P, DH, 4], FP16)
    nc.vector.tensor_copy(vecs16[:, :, 0:3], vecsT[:])

    w12t16 = weights.tile([P, FT, F], FP16)  # [fl, k, f'] = W12[f', k*128+fl]
    nc.sync.dma_start(w12t16[:], w12t_d.rearrange("(k p) f -> p k f", p=P))
    b12bc16 = const.tile([P, F], FP16)  # b12 broadcast to all partitions
    nc.gpsimd.dma_start(
        out=b12bc16[:],
        in_=bass.AP(tensor=b12r_d.tensor, offset=b12r_d.offset,
                    ap=[[0, P]] + b12r_d.ap[1:]),
    )
    mT = const.tile([P, n_elems * CT], FP32)  # [p, b*8+t] = mask[b, t*128+p]
    nc.sync.dma_start(mT[:], mT_d)

    if reps > 1:
        es.enter_context(tc.For_i(0, reps, 1))

    def preamble(cn, qn, cnat16, idx):
        """Transposes + fp16 prep: ctxT16, qst16, qstT16, qmx16, qwqx16."""
        ctxT16 = work.tile([P, DH, C], FP16, tag="ctxT16", bufs=3,
                           name=f"ctxT16_{idx}")
        for dh in range(DH):
            for g in range(2):  # two groups of 4 c-tiles -> one psum bank
                pt = psA.tile([P, 512], FP16, tag="ps_mm", name=f"ptc{idx}{dh}{g}")
                for j in range(4):
                    t = g * 4 + j
                    nc.tensor.transpose(
                        pt[:, j * P:(j + 1) * P],
                        cnat16[:, t, dh * P:(dh + 1) * P],
                        ident16[:],
                    )
                nc.vector.tensor_copy(ctxT16[:, dh, g * 512:(g + 1) * 512], pt[:])

        qst16 = work.tile([P, D], FP16, tag="qst16", bufs=2, name=f"qst16_{idx}")
        nc.vector.tensor_copy(qst16[:], qn[:])

        pq = psB.tile([P, 2 * P], FP16, tag="ps_small", name=f"pq{idx}")
        for dh in range(DH):
            nc.tensor.transpose(pq[:, dh * P:(dh + 1) * P],
                                qst16[:, dh * P:(dh + 1) * P], ident16[:])
        qstT16 = work.tile([P, DH, P], FP16, tag="qstT16", bufs=2,
                           name=f"qstT16_{idx}")
        nc.vector.tensor_copy(qstT16[:].rearrange("p h q -> p (h q)"), pq[:])

        # moving operand of sim: [q*wm | w_c], padded to 136 for alignment
        qmx16 = work.tile([P, DH, Q + 8], FP16, tag="qmx16", bufs=2,
                          name=f"qmx16_{idx}")
        for dh in range(DH):
            nc.vector.tensor_scalar_mul(qmx16[:, dh, 0:Q], qstT16[:, dh, :],
                                        vecsT[:, dh, 2:3])
            nc.vector.tensor_copy(qmx16[:, dh, Q:Q + 1], vecsT[:, dh, 1:2])

        # qwq row (K=1 accumulate into sim); col Q is 0 so cwc stays clean
        pw = psB.tile([1, P], FP32, tag="ps_small", name=f"pw{idx}")
        for dh in range(DH):
            nc.tensor.matmul(pw[:], vecs16[:, dh, 0:1], qstT16[:, dh, :],
                             start=(dh == 0), stop=(dh == DH - 1))
        qwqx16 = work.tile([1, Q + 8], FP16, tag="qwqx16", bufs=2,
                           name=f"qwqx16_{idx}")
        nc.vector.tensor_copy(qwqx16[0:1, 0:Q], pw[:])
        nc.vector.memset(qwqx16[0:1, Q:Q + 1], 0.0)
        return ctxT16, qst16, qstT16, qmx16, qwqx16

    def emit_fused(stage, cts):
        """Fused layer for `stage`'s elem (natural layout) + relu*mask +
        store.  Emitted interleaved into the NEXT elem's attention phase so
        its matmuls fill PE during that elem's serial q2c chain.  Piece
        order puts the late-arriving W_eff operands last."""
        if stage is None:
            return
        fb, fctxT16, fcxc16, fPT16, fQB16, fweff16 = stage
        for ct in cts:
            sl = slice(ct * P, (ct + 1) * P)
            pieces = [
                (fcxc16[:, 0, sl], w12t16[:, 4, :]),
                (fcxc16[:, 1, sl], w12t16[:, 5, :]),
                (fPT16[:, sl], fQB16[:]),
                (fctxT16[:, 0, sl], fweff16[:, 0, :]),
                (fctxT16[:, 1, sl], fweff16[:, 1, :]),
            ]
            p2 = [psA.tile([P, 512], FP32, tag="ps_mm", name=f"p2_{fb}{ct}{fh}")
                  for fh in range(2)]
            npc = len(pieces)
            for i, (lh, rh) in enumerate(pieces):
                for fh in range(2):
                    nc.tensor.matmul(
                        p2[fh][:], lh, rh[:, fh * 512:(fh + 1) * 512],
                        start=(i == 0), stop=(i == npc - 1),
                    )
            osb = outp.tile([P, F], FP16, tag="osb")
            for fh in range(2):
                nc.scalar.activation(
                    osb[:, fh * 512:(fh + 1) * 512], p2[fh][:], AF.Relu,
                    scale=mT[:, fb * CT + ct:fb * CT + ct + 1],
                )
            nc.sync.dma_start(out_d[fb, sl, :], osb[:])

    pre = None
    prev_stage = None
    for b in range(n_elems):
        # ---- loads (elem b prefetched; prefetch b+1 now) ----
        if pend is None:
            pend = load_elem(b, b)
        cn, qn, cnat16 = pend
        pend = load_elem(b + 1, b + 1) if b + 1 < n_elems else None
        if pre is None:
            pre = preamble(cn, qn, cnat16, b)
        ctxT16, qst16, qstT16, qmx16, qwqx16 = pre

        # ---- sim tiles + softmax over q (free dim) ----
        nmx = work.tile([P, CT], FP32, tag="nmx")    # negated row max
        sume = work.tile([P, CT], FP32, tag="sume")
        rs = work.tile([P, CT], FP32, tag="rs")
        pcwc = work.tile([P, CT], FP32, tag="pcwc")  # cwc columns [c_l, t]
        Pm16 = work.tile([P, CT, Q], FP16, tag="Pm16")
        for t in range(CT):
            ps = psB.tile([P, Q + 1], FP32, tag="ps_small")
            for dh in range(DH):
                nc.tensor.matmul(
                    ps[:], ctxT16[:, dh, t * P:(t + 1) * P],
                    qmx16[:, dh, 0:Q + 1],
                    start=(dh == 0), stop=False,
                )
            nc.tensor.matmul(ps[:], ones16[:], qwqx16[:, 0:Q + 1],
                             start=False, stop=True)
            nc.vector.reduce_max(nmx[:, t:t + 1], ps[:, 0:Q], axis=AX,
                                 negate=True)
            nc.scalar.activation(
                Pm16[:, t, :], ps[:, 0:Q], AF.Exp, bias=nmx[:, t:t + 1],
                accum_out=sume[:, t:t + 1],
            )
            nc.vector.tensor_copy(pcwc[:, t:t + 1], ps[:, Q:Q + 1])
            nc.vector.reciprocal(rs[:, t:t + 1], sume[:, t:t + 1])
            nc.vector.tensor_scalar_mul(Pm16[:, t, :], Pm16[:, t, :],
                                        rs[:, t:t + 1])

        # ---- q2c chain start (DVE side; its PE ops are interleaved below
        # so independent PT/c2q/QB matmuls fill PE during engine handoffs) --
        madj = work.tile([P, CT], FP32, tag="madj")  # m_c = cwc - nmx
        nc.vector.tensor_sub(madj[:], pcwc[:], nmx[:])
        colmin = work.tile([P, 1], FP32, tag="colmin")
        nc.vector.reduce_max(colmin[:], madj[:], axis=AX, negate=True)

        # ---- PT, c2q -> cxc (chain PE ops interleaved) ----
        PT16 = work.tile([P, C], FP16, tag="PT16", bufs=2, name=f"PT16_{b}")
        cxc16 = work.tile([P, DH, C], FP16, tag="cxc16", bufs=2,
                          name=f"cxc16_{b}")

        def pt_c2q_group(g):
            pt16 = psA.tile([P, 512], FP16, tag="ps_mm", name=f"pt16{g}")
            for j in range(4):
                t = g * 4 + j
                nc.tensor.transpose(pt16[:, j * P:(j + 1) * P], Pm16[:, t, :],
                                    ident16[:])
            nc.vector.tensor_copy(PT16[:, g * 512:(g + 1) * 512], pt16[:])
            for dh in range(DH):
                pc2 = psA.tile([P, 512], FP32, tag="ps_mm", name=f"pc2{g}{dh}")
                nc.tensor.matmul(
                    pc2[:], qst16[:, dh * P:(dh + 1) * P],
                    PT16[:, g * 512:(g + 1) * 512],
                    start=True, stop=True,
                )
                nc.vector.tensor_mul(cxc16[:, dh, g * 512:(g + 1) * 512],
                                     ctxT16[:, dh, g * 512:(g + 1) * 512],
                                     pc2[:])

        pt_c2q_group(0)
        pcm = psB.tile([1, P], FP32, tag="ps_small")
        nc.tensor.transpose(pcm[:], colmin[:], ident[:])
        minall = work.tile([1, 2], FP32, tag="minall")
        nc.vector.tensor_reduce(minall[:, 0:1], pcm[:], axis=AX,
                                op=mybir.AluOpType.min)
        pt_c2q_group(1)
        pmb = psB.tile([P, 1], FP32, tag="ps_small")
        nc.tensor.matmul(pmb[:], ones_row[:], minall[:, 0:1], start=True,
                         stop=True)
        minb = work.tile([P, 1], FP32, tag="minb")
        nc.vector.tensor_copy(minb[:], pmb[:])
        wall16 = work.tile([P, CT], FP16, tag="wall16")  # exp(m - Mglob)
        nc.scalar.activation(wall16[:], madj[:], AF.Exp, bias=minb[:])

        # ---- QB' = q@B.T + b12 (PE fill while wall16 lands) ----
        QB16 = work.tile([P, F], FP16, tag="QB16", bufs=2, name=f"QB16_{b}")
        for fh in range(2):
            pqb = psA.tile([P, 512], FP32, tag="ps_mm", name=f"pqb{fh}")
            for dh in range(DH):
                nc.tensor.matmul(
                    pqb[:], qstT16[:, dh, :],
                    w12t16[:, 2 + dh, fh * 512:(fh + 1) * 512],
                    start=(dh == 0), stop=(dh == DH - 1),
                )
            nc.vector.tensor_add(QB16[:, fh * 512:(fh + 1) * 512], pqb[:],
                                 b12bc16[:, fh * 512:(fh + 1) * 512])

        emit_fused(prev_stage, range(0, 3))  # PE fill while wall16 lands

        # ---- q2c numerator/denominator + W_eff = A + D*diag(q2c) ----
        pn = psB.tile([1, D + 1], FP32, tag="ps_small", name="pn")
        for t in range(CT):
            nc.tensor.matmul(pn[:], wall16[:, t:t + 1], cnat16[:, t, 0:D + 1],
                             start=(t == 0), stop=(t == CT - 1))
        rden = work.tile([1, 1], FP32, tag="rden")
        nc.vector.reciprocal(rden[:], pn[0:1, D:D + 1])
        q2cr = work.tile([1, D], FP32, tag="q2cr")
        nc.vector.tensor_scalar_mul(q2cr[:], pn[0:1, 0:D], rden[:])
        emit_fused(prev_stage, range(3, 4))  # PE fill while q2cr lands
        q2cc = work.tile([P, DH], FP32, tag="q2cc")  # [d_l, dh]
        for dh in range(DH):
            pq2 = psB.tile([P, 1], FP32, tag="ps_small", name=f"pq2{dh}")
            nc.tensor.transpose(pq2[:], q2cr[0:1, dh * P:(dh + 1) * P],
                                ident[0:1, 0:1])
            nc.vector.tensor_copy(q2cc[:, dh:dh + 1], pq2[:])
        emit_fused(prev_stage, range(4, 5))  # PE fill while W_eff preps
        weff16 = work.tile([P, DH, F], FP16, tag="weff16", bufs=2,
                           name=f"weff16_{b}")
        for k in range(DH):
            nc.vector.tensor_scalar_mul(weff16[:, k, :], w12t16[:, 6 + k, :],
                                        q2cc[:, k:k + 1])
            nc.vector.tensor_add(weff16[:, k, :], weff16[:, k, :],
                                 w12t16[:, k, :])

        # next elem's preamble: more PE fill for the tail of the chain
        pre = preamble(pend[0], pend[1], pend[2], b + 1) if pend else None

        stage = (b, ctxT16, cxc16, PT16, QB16, weff16)
        emit_fused(prev_stage, range(5, CT))  # finish elem b-1
        prev_stage = stage

    emit_fused(prev_stage, range(CT))  # drain: last elem's fused layer


_NC_CACHE = {}


def _build_nc(n_elems=BPC, reps=1):
    key = (n_elems, reps)
    if key in _NC_CACHE:
        return _NC_CACHE[key]
    nc = bacc.Bacc("TRN2", target_bir_lowering=False, debug=False,
                   num_devices=NCORES)
    ins = [
        nc.dram_tensor("ctx", (n_elems, C, D), FP32, kind="ExternalInput").ap(),
        nc.dram_tensor("qst", (n_elems, Q, D), FP32, kind="ExternalInput").ap(),
        nc.dram_tensor("vecsT", (D, 3), FP32, kind="ExternalInput").ap(),
        nc.dram_tensor("w12t", (F, F), FP16, kind="ExternalInput").ap(),
        nc.dram_tensor("b12r", (1, F), FP16, kind="ExternalInput").ap(),
        nc.dram_tensor("mT", (P, n_elems * CT), FP32, kind="ExternalInput").ap(),
    ]
    outs = [nc.dram_tensor("out", (n_elems, C, F), FP16,
                           kind="ExternalOutput").ap()]
    from contextlib import ExitStack
    with tile.TileContext(nc) as tc, ExitStack() as es:
        _build_body(es, tc, outs, ins, n_elems=n_elems, reps=reps)
    nc.compile()
    _NC_CACHE[key] = (nc, ins, outs)
    return _NC_CACHE[key]


def _host_prep(context, question, context_mask, w_question, w_context,
               w_multiple, W1, b1, W2, b2):
    """Build the 8 per-core input maps from full inputs."""
    context = np.asarray(context, np.float32)
    question = np.asarray(question, np.float32)
    maskf = np.asarray(context_mask).astype(np.float32)
    W1f = np.asarray(W1, np.float32)
    W2f = np.asarray(W2, np.float32)
    W12 = W2f @ W1f
    b12 = W2f @ np.asarray(b1, np.float32) + np.asarray(b2, np.float32)
    vecsT = np.ascontiguousarray(
        np.stack([w_question, w_context, w_multiple]).T.astype(np.float32))
    w12t16 = np.ascontiguousarray(W12.T.astype(np.float16))  # [f, f']
    b12r16 = b12.reshape(1, F).astype(np.float16)
    in_maps = []
    for i in range(NCORES):
        sl = slice(BPC * i, BPC * (i + 1))
        mTc = np.ascontiguousarray(
            maskf[sl].reshape(BPC, CT, P).transpose(2, 0, 1).reshape(P, BPC * CT))
        in_maps.append({
            "ctx": np.ascontiguousarray(context[sl]),
            "qst": np.ascontiguousarray(question[sl]),
            "vecsT": vecsT,
            "w12t": w12t16,
            "b12r": b12r16,
            "mT": mTc,
        })
    return in_maps


def kernel(context, question, context_mask, w_question, w_context, w_multiple,
           W1, b1, W2, b2):
    nc, _, _ = _build_nc()
    in_maps = _host_prep(context, question, context_mask, w_question,
                         w_context, w_multiple, W1, b1, W2, b2)
    res = run_bass_kernel_spmd(nc, in_maps, list(range(NCORES))).results
    out = np.concatenate([res[i]["out"] for i in range(NCORES)], axis=0)
    return out.astype(np.float32)
